# revision 1
# baseline (speedup 1.0000x reference)
"""2-layer GraphConv GNN on 8 trn2 NeuronCores (Bass/Tile).

Strategy (hardcoded for N=100000 nodes, E=1600000 edges, F=128, H=128, O=64):
  - Shard edges by destination node: core c owns dst in [c*12500, (c+1)*12500).
  - Aggregation via PE matmul segment-sum: edges chunked 128 at a time;
    msgs [128 edges, 128 feat] (bf16, gathered via dma_gather) as lhsT,
    one-hot S [128 edges, 128 dst-slots] (built on DVE via iota==dst compare)
    as rhs; accumulate into PSUM [128 feat, 128 dst] per 128-dst group.
  - Gather: dma_gather (int16 idx) with sources split into 4 ranges of 25000
    rows; 4 SWDGE queues in parallel. Edges laid out in slots grouped by
    (supergroup, src-range, dst-group), dst-sorted, padded to fixed budgets
    (SPMD-uniform across cores; pad idx = -1 -> skipped).
  - Layer transforms on PE from feature-major agg + host-pretransposed x/w.
  - Inter-layer exchange: AllGather of h (bf16) across the 8 cores.
"""

import numpy as np
import ml_dtypes
from contextlib import ExitStack

N = 100000
F = 128          # input/hidden feature dim
O = 64           # output dim
NC = 8
SHARD = N // NC          # 12500
G = 128                  # dst nodes per psum group
NGROUP = (SHARD + G - 1) // G   # 98 (last group has 84 nodes)
LASTG = SHARD - (NGROUP - 1) * G  # 84
NR = 4                   # src ranges (int16 gather index limit)
RS = N // NR             # 25000
SB = 640                 # slot budget per (group, range); 5 chunks of 128
CHUNKS_PER_SEG = SB // 128  # 5
SG_SIZE = 4              # groups per supergroup (gather call batching)

bf16 = ml_dtypes.bfloat16


def _supergroups():
    sgs = []
    g0 = 0
    while g0 < NGROUP:
        sgs.append(list(range(g0, min(g0 + SG_SIZE, NGROUP))))
        g0 += SG_SIZE
    return sgs


SGS = _supergroups()
NCHUNKS = NGROUP * NR * CHUNKS_PER_SEG  # 1960 chunks per layer
TOTSLOTS = NGROUP * NR * SB             # 250880


def _prep_core(src, dst_local):
    """Slot layout for one core. Returns idx16 [128, TOTSLOTS//16] (int16,
    per-call 16-wrapped+replicated) and dstS [128, NCHUNKS] bf16."""
    g = dst_local // G
    r = src // RS
    bucket = g * NR + r
    order = np.lexsort((dst_local, bucket))
    s_o = src[order]
    d_o = dst_local[order]
    b_o = bucket[order]
    cnt = np.bincount(b_o, minlength=NGROUP * NR)
    if cnt.max() > SB:
        raise RuntimeError(f"bucket overflow: {cnt.max()} > {SB}")

    # slot base per bucket in (sg, r, g_local) call-major order
    slotbase = np.zeros(NGROUP * NR, dtype=np.int64)
    pos = 0
    for sg in SGS:
        for r_ in range(NR):
            for g_ in sg:
                slotbase[g_ * NR + r_] = pos
                pos += SB
    start = np.zeros(NGROUP * NR + 1, dtype=np.int64)
    np.cumsum(cnt, out=start[1:])
    within = np.arange(len(b_o)) - start[b_o]
    slot = slotbase[b_o] + within

    # pad slots gather row 0 of the range (S row is zero, so value unused).
    # (-1 "skip" semantics are only safe for trailing pads; ours are interior.)
    idx_val = np.zeros(TOTSLOTS, dtype=np.int16)
    idx_val[slot] = (s_o - (s_o // RS) * RS).astype(np.int16)
    dst_val = np.full(TOTSLOTS, 200, dtype=np.float32)  # pad: no iota match
    dst_val[slot] = (d_o % G).astype(np.float32)

    # per-call 16-wrap: call = (sg, r) covering len(sg)*SB slots
    cols = []
    pos = 0
    for sg in SGS:
        ncall = len(sg) * SB
        for r_ in range(NR):
            blk = idx_val[pos : pos + ncall]
            cols.append(blk.reshape(ncall // 16, 16).T)  # [16, ncall/16]
            pos += ncall
    idx16 = np.tile(np.concatenate(cols, axis=1), (8, 1))  # [128, TOTSLOTS/16]

    dstS = np.ascontiguousarray(dst_val.reshape(NCHUNKS, 128).T)  # [128, NCHUNKS] f32
    return idx16, dstS


import os
_L1ONLY = bool(int(os.environ.get("GNN_L1ONLY", "0")))


def _build_program():
    import concourse.bass as bass
    import concourse.tile as tile
    from concourse import bacc, mybir

    nc = bacc.Bacc(None, target_bir_lowering=False, num_swdge_queues=4)
    dt = mybir.dt

    # inputs
    xbf = nc.dram_tensor("xbf", [N, F], dt.bfloat16, kind="ExternalInput")
    xiT = nc.dram_tensor("xiT", [F, SHARD], dt.float32, kind="ExternalInput")
    idx16 = nc.dram_tensor("idx16", [128, TOTSLOTS // 16], dt.int16, kind="ExternalInput")
    dstS_in = nc.dram_tensor("dstS", [128, NCHUNKS], dt.float32, kind="ExternalInput")
    wr1T = nc.dram_tensor("wr1T", [F, F], dt.float32, kind="ExternalInput")
    wo1T = nc.dram_tensor("wo1T", [F, F], dt.float32, kind="ExternalInput")
    wr2T = nc.dram_tensor("wr2T", [F, O], dt.float32, kind="ExternalInput")
    wo2T = nc.dram_tensor("wo2T", [F, O], dt.float32, kind="ExternalInput")
    b1_in = nc.dram_tensor("b1", [1, F], dt.float32, kind="ExternalInput")
    b2_in = nc.dram_tensor("b2", [1, O], dt.float32, kind="ExternalInput")
    iota_in = nc.dram_tensor("iota", [128, G], dt.bfloat16, kind="ExternalInput")
    iota32_in = nc.dram_tensor("iota32", [128, G], dt.float32, kind="ExternalInput")
    ident_in = nc.dram_tensor("ident", [128, 128], dt.float32, kind="ExternalInput")
    ones_in = nc.dram_tensor("ones", [1, G], dt.float32, kind="ExternalInput")
    out_t = nc.dram_tensor("out", [SHARD, O], dt.float32, kind="ExternalOutput")

    # internal DRAM: exchange hr = h @ w_rel2.T (64-wide) instead of h
    hr_shard = nc.dram_tensor("hr_shard", [SHARD, O], dt.bfloat16)
    hr_full_bf = nc.dram_tensor("hr_full_bf", [N, O], dt.bfloat16, addr_space="Shared")
    hr_full = nc.dram_tensor("hr_full", [N, O], dt.float32)

    with tile.TileContext(nc) as tc, ExitStack() as ctx:
        const_p = ctx.enter_context(tc.tile_pool(name="const", bufs=1))
        resid_p = ctx.enter_context(tc.tile_pool(name="resid", bufs=1))
        idx_p = ctx.enter_context(tc.tile_pool(name="idxp", bufs=8))
        msgs_p = ctx.enter_context(tc.tile_pool(name="msgs", bufs=8))
        s_p = ctx.enter_context(tc.tile_pool(name="sp", bufs=8))
        agg_p = ctx.enter_context(tc.tile_pool(name="aggp", bufs=3))
        hsb_p = ctx.enter_context(tc.tile_pool(name="hsb", bufs=3))
        osb_p = ctx.enter_context(tc.tile_pool(name="osb", bufs=3))
        ps_agg = ctx.enter_context(tc.tile_pool(name="ps_agg", bufs=2, space="PSUM"))
        ps_h = ctx.enter_context(tc.tile_pool(name="ps_h", bufs=2, space="PSUM"))
        ps_t = ctx.enter_context(tc.tile_pool(name="ps_t", bufs=1, space="PSUM"))

        # constants / residents
        c_iota = const_p.tile([128, G], dt.bfloat16)
        nc.sync.dma_start(c_iota[:], iota_in[:])
        c_iota32 = const_p.tile([128, G], dt.float32)
        nc.sync.dma_start(c_iota32[:], iota32_in[:])
        c_ident = const_p.tile([128, 128], dt.float32)
        nc.sync.dma_start(c_ident[:], ident_in[:])
        c_ones = const_p.tile([1, G], dt.float32)
        nc.sync.dma_start(c_ones[:], ones_in[:])
        c_wr1T = const_p.tile([F, F], dt.float32)
        nc.sync.dma_start(c_wr1T[:], wr1T[:])
        c_wo1T = const_p.tile([F, F], dt.float32)
        nc.sync.dma_start(c_wo1T[:], wo1T[:])
        c_wr2T = const_p.tile([F, O], dt.float32)
        nc.sync.dma_start(c_wr2T[:], wr2T[:])
        c_wo2T = const_p.tile([F, O], dt.float32)
        nc.sync.dma_start(c_wo2T[:], wo2T[:])
        c_b1 = const_p.tile([1, F], dt.float32)
        nc.sync.dma_start(c_b1[:], b1_in[:])
        c_b2 = const_p.tile([1, O], dt.float32)
        nc.sync.dma_start(c_b2[:], b2_in[:])
        c_dstS = const_p.tile([128, NCHUNKS], dt.float32)
        nc.sync.dma_start(c_dstS[:], dstS_in[:])
        r_xiT = resid_p.tile([F, SHARD], dt.float32)
        nc.sync.dma_start(r_xiT[:], xiT[:])
        r_hT = resid_p.tile([F, SHARD], dt.float32)  # written in L1, read in L2

        def layer(L):
            """L=1: table=xbf, produce h (hT resident + h_shard DRAM).
            L=2: table=h_full, produce out."""
            table = xbf if L == 1 else hr_full
            call_idx = 0   # column offset into idx16 (units of 16-wrapped cols)
            chunk_idx = 0  # global chunk counter (dstS column)
            for sg in SGS:
                ng = len(sg)
                call_slots = ng * SB
                call_cols = call_slots // 16
                blocks = call_slots // 128
                msgs = []
                for r_ in range(NR):
                    it = idx_p.tile([128, call_cols], dt.int16, tag="idx")
                    nc.sync.dma_start(
                        it[:], idx16[:, call_idx : call_idx + call_cols]
                    )
                    FW = F if L == 1 else O
                    mdt = dt.bfloat16 if L == 1 else dt.float32
                    m = msgs_p.tile([128, blocks * FW], mdt, tag="m" + str(L))
                    nc.gpsimd.dma_gather(
                        m[:].rearrange("p (c e) -> p c e", e=FW),
                        table[r_ * RS : (r_ + 1) * RS, :],
                        it[:],
                        call_slots,
                        call_slots,
                        FW,
                        single_packet=False,
                        queue_num=r_,
                    )
                    msgs.append(m)
                    call_idx += call_cols
                for gl, g_ in enumerate(sg):
                    ngn = G if g_ < NGROUP - 1 else LASTG
                    gbase = g_ * G
                    psum = ps_agg.tile([128, G], dt.float32, tag="agg", space="PSUM")
                    nmm = NR * CHUNKS_PER_SEG
                    mm = 0
                    for r_ in range(NR):
                        for k in range(CHUNKS_PER_SEG):
                            b = gl * CHUNKS_PER_SEG + k
                            # chunk index in slot layout: (sg, r, g_local, k)
                            ci = chunk_idx + (r_ * ng + gl) * CHUNKS_PER_SEG + k
                            sdt = dt.bfloat16 if L == 1 else dt.float32
                            S = s_p.tile([128, G], sdt, tag="S" + str(L))
                            nc.vector.tensor_scalar(
                                out=S[:],
                                in0=c_iota[:] if L == 1 else c_iota32[:],
                                scalar1=c_dstS[:, ci : ci + 1],
                                scalar2=None,
                                op0=mybir.AluOpType.is_equal,
                            )
                            FW = F if L == 1 else O
                            nc.tensor.matmul(
                                psum[:FW, :],
                                lhsT=msgs[r_][:, b * FW : (b + 1) * FW],
                                rhs=S[:],
                                start=(mm == 0),
                                stop=(mm == nmm - 1),
                            )
                            mm += 1
                    FW = F if L == 1 else O
                    aggT = agg_p.tile([128, G], dt.float32, tag="aggT")
                    nc.scalar.copy(out=aggT[:FW, :], in_=psum[:FW, :])
                    if L == 1:
                        ph = ps_h.tile([128, G], dt.float32, tag="ph", space="PSUM")
                        nc.tensor.matmul(ph[:], lhsT=c_wr1T[:], rhs=aggT[:], start=True, stop=False)
                        nc.tensor.matmul(ph[:, :ngn], lhsT=c_wo1T[:], rhs=r_xiT[:, gbase : gbase + ngn], start=False, stop=False)
                        nc.tensor.matmul(ph[:, :ngn], lhsT=c_b1[:1, :], rhs=c_ones[:1, :ngn], start=False, stop=True)
                        # relu -> hT resident (fp32)
                        nc.scalar.activation(
                            out=r_hT[:, gbase : gbase + ngn],
                            in_=ph[:, :ngn],
                            func=mybir.ActivationFunctionType.Relu,
                        )
                        # hrT = w_rel2.T-transform of hT slice (feature-major)
                        phr = ps_t.tile([128, 128], dt.float32, tag="phr", space="PSUM")
                        nc.tensor.matmul(phr[:O, :ngn], lhsT=c_wr2T[:], rhs=r_hT[:, gbase : gbase + ngn], start=True, stop=True)
                        hrT = hsb_p.tile([128, G], dt.float32, tag="hrT")
                        nc.scalar.copy(out=hrT[:O, :ngn], in_=phr[:O, :ngn])
                        # transpose -> node-major hr (bf16) -> DRAM for AllGather
                        pt = ps_t.tile([128, 128], dt.float32, tag="pt", space="PSUM")
                        nc.tensor.transpose(pt[:ngn, :O], hrT[:O, :ngn], c_ident[:O, :O])
                        hsb = hsb_p.tile([128, O], dt.bfloat16, tag="hsb")
                        nc.scalar.copy(out=hsb[:ngn, :], in_=pt[:ngn, :O])
                        nc.sync.dma_start(hr_shard[gbase : gbase + ngn, :], hsb[:ngn, :])
                    else:
                        po = ps_h.tile([128, O], dt.float32, tag="po", space="PSUM")
                        # agg2 already rel2-transformed: just transpose to node-major
                        nc.tensor.matmul(po[:ngn, :], lhsT=aggT[:O, :ngn], rhs=c_ident[:O, :O], start=True, stop=False, is_transpose=True)
                        nc.tensor.matmul(po[:ngn, :], lhsT=r_hT[:, gbase : gbase + ngn], rhs=c_wo2T[:], start=False, stop=False)
                        nc.tensor.matmul(po[:ngn, :], lhsT=c_ones[:1, :ngn], rhs=c_b2[:1, :], start=False, stop=True)
                        osb = osb_p.tile([128, O], dt.float32, tag="osb")
                        nc.scalar.copy(out=osb[:ngn, :], in_=po[:ngn, :])
                        nc.sync.dma_start(out_t[gbase : gbase + ngn, :], osb[:ngn, :])
                chunk_idx += ng * NR * CHUNKS_PER_SEG

        layer(1)
        if _L1ONLY:
            zo = osb_p.tile([128, O], dt.float32, tag="osb")
            nc.vector.memset(zo[:], 0.0)
            nc.sync.dma_start(out_t[0:128, :], zo[:])
        else:
            nc.gpsimd.collective_compute(
                "AllGather",
                mybir.AluOpType.bypass,
                replica_groups=[list(range(NC))],
                ins=[hr_shard[:]],
                outs=[hr_full_bf[:]],
            )
            # expand bf16 -> fp32 (cast-DMA, DRAM->DRAM) so L2 gathers 256-B rows
            flat_bf = hr_full_bf[:].rearrange("n o -> (n o)").rearrange("(a b) -> a b", a=128)
            flat_f32 = hr_full[:].rearrange("n o -> (n o)").rearrange("(a b) -> a b", a=128)
            CW = flat_bf.shape[1]
            step = CW // 10
            for i in range(10):
                lo, hi = i * step, (i + 1) * step if i < 9 else CW
                nc.gpsimd.dma_start(flat_f32[:, lo:hi], flat_bf[:, lo:hi])
            layer(2)

    nc.finalize()
    return nc


_CACHED = {}


def prepare_in_maps(inputs):
    x = np.asarray(inputs["x"], dtype=np.float32)
    edge_index = np.asarray(inputs["edge_index"])
    w_rel1 = np.asarray(inputs["w_rel1"], dtype=np.float32)
    b_rel1 = np.asarray(inputs["b_rel1"], dtype=np.float32)
    w_root1 = np.asarray(inputs["w_root1"], dtype=np.float32)
    w_rel2 = np.asarray(inputs["w_rel2"], dtype=np.float32)
    b_rel2 = np.asarray(inputs["b_rel2"], dtype=np.float32)
    w_root2 = np.asarray(inputs["w_root2"], dtype=np.float32)

    src = edge_index[0].astype(np.int64)
    dst = edge_index[1].astype(np.int64)

    xbf = x.astype(bf16)
    iota = np.broadcast_to(np.arange(G, dtype=np.float32), (128, G)).astype(bf16)
    ident = np.eye(128, dtype=np.float32)
    ones = np.ones((1, G), dtype=np.float32)

    in_maps = []
    for c in range(NC):
        m = (dst >= c * SHARD) & (dst < (c + 1) * SHARD)
        idx16, dstS = _prep_core(src[m], dst[m] - c * SHARD)
        in_maps.append(
            {
                "xbf": xbf,
                "xiT": np.ascontiguousarray(x[c * SHARD : (c + 1) * SHARD, :].T),
                "idx16": idx16,
                "dstS": dstS,
                "wr1T": np.ascontiguousarray(w_rel1.T),
                "wo1T": np.ascontiguousarray(w_root1.T),
                "wr2T": np.ascontiguousarray(w_rel2.T),
                "wo2T": np.ascontiguousarray(w_root2.T),
                "b1": b_rel1.reshape(1, F),
                "b2": b_rel2.reshape(1, O),
                "iota": iota,
                "iota32": np.broadcast_to(np.arange(G, dtype=np.float32), (128, G)).copy(),
                "ident": ident,
                "ones": ones,
            }
        )
    return in_maps


def get_nc():
    if "nc" not in _CACHED:
        _CACHED["nc"] = _build_program()
    return _CACHED["nc"]


def kernel(**inputs):
    from concourse.bass_utils import run_bass_kernel_spmd

    in_maps = prepare_in_maps(inputs)
    nc = get_nc()
    res = run_bass_kernel_spmd(nc, in_maps, core_ids=list(range(NC)), trace=False)
    out = np.concatenate([res.results[c]["out"] for c in range(NC)], axis=0)
    return out.astype(np.float32)



# revision 37
# speedup vs baseline: 425.4583x; 425.4583x over previous
"""2-layer GraphConv GNN on 8 trn2 NeuronCores (Bass/Tile).

Strategy (hardcoded for N=100000 nodes, E=1600000 edges, F=128, H=128, O=64):
  - Shard edges by destination node: core c owns dst in [c*12500, (c+1)*12500).
  - Aggregation via PE matmul segment-sum: edges chunked 128 at a time;
    msgs [128 edges, 128 feat] (bf16, gathered via dma_gather) as lhsT,
    one-hot S [128 edges, 128 dst-slots] (built on DVE via iota==dst compare)
    as rhs; accumulate into PSUM [128 feat, 128 dst] per 128-dst group.
  - Gather: dma_gather (int16 idx) with sources split into 4 ranges of 25000
    rows; 4 SWDGE queues in parallel. Edges laid out in slots grouped by
    (supergroup, src-range, dst-group), dst-sorted, padded to fixed budgets
    (SPMD-uniform across cores; pad idx = -1 -> skipped).
  - Layer transforms on PE from feature-major agg + host-pretransposed x/w.
  - Inter-layer exchange: AllGather of h (bf16) across the 8 cores.
"""

import os
import numpy as np
import ml_dtypes
from contextlib import ExitStack

N = 100000
F = 128          # input/hidden feature dim
O = 64           # output dim
NC = 8
SHARD = N // NC          # 12500
GW = int(os.environ.get("GNN_GW", "128"))  # dst nodes per psum group
NGROUP = (SHARD + GW - 1) // GW
LASTG = SHARD - (NGROUP - 1) * GW
NR = 4                   # src ranges (int16 gather index limit)
RS = N // NR             # 25000
# slot budget per (group, range) bucket; overridden adaptively from the
# actual per-bucket edge counts in prepare_in_maps (max over the 8 cores,
# rounded up to a multiple of 128).
SB = int(os.environ.get("GNN_SB", "640" if GW == 128 else "1280"))
SG_SIZE = int(os.environ.get("GNN_SG", "4" if GW == 128 else "2"))
MBUFS = int(os.environ.get("GNN_MBUFS", "8"))
PSBUFS = int(os.environ.get("GNN_PSBUFS", "2"))
IBUFS = int(os.environ.get("GNN_IBUFS", "8"))
SCRATCH = int(os.environ.get("GNN_SCRATCH", "16384"))

bf16 = ml_dtypes.bfloat16


def _recalc():
    global CHUNKS_PER_SEG, SGS, NCHUNKS, TOTSLOTS
    CHUNKS_PER_SEG = SB // 128
    sgs = []
    g0 = 0
    while g0 < NGROUP:
        sgs.append(list(range(g0, min(g0 + SG_SIZE, NGROUP))))
        g0 += SG_SIZE
    SGS = sgs
    NCHUNKS = NGROUP * NR * CHUNKS_PER_SEG
    TOTSLOTS = NGROUP * NR * SB


_recalc()


def _set_budget(max_bucket_count):
    """Adapt SB to the data (shared across cores; program is SPMD)."""
    global SB
    want = max(128, -(-int(max_bucket_count) // 128) * 128)
    if want != SB:
        SB = want
        _recalc()
        _CACHED.clear()


def _prep_core(src, dst_local):
    """Slot layout for one core. Returns idx16 [128, TOTSLOTS//16] (int16,
    per-call 16-wrapped+replicated) and dstS [128, NCHUNKS] f32."""
    g = dst_local // GW
    r = src // RS
    bucket = g * NR + r
    order = np.lexsort((dst_local, bucket))
    s_o = src[order]
    d_o = dst_local[order]
    b_o = bucket[order]
    cnt = np.bincount(b_o, minlength=NGROUP * NR)
    if cnt.max() > SB:
        raise RuntimeError(f"bucket overflow: {cnt.max()} > {SB}")

    # slot base per bucket in (sg, r, g_local) call-major order
    slotbase = np.zeros(NGROUP * NR, dtype=np.int64)
    pos = 0
    for sg in SGS:
        for r_ in range(NR):
            for g_ in sg:
                slotbase[g_ * NR + r_] = pos
                pos += SB
    start = np.zeros(NGROUP * NR + 1, dtype=np.int64)
    np.cumsum(cnt, out=start[1:])
    within = np.arange(len(b_o)) - start[b_o]
    slot = slotbase[b_o] + within

    # pad slots gather row 0 of the range (S row is zero, so value unused).
    # (-1 "skip" semantics are only safe for trailing pads; ours are interior.)
    idx_val = np.zeros(TOTSLOTS, dtype=np.int16)
    idx_val[slot] = (s_o - (s_o // RS) * RS).astype(np.int16)
    dst_val = np.full(TOTSLOTS, GW + 72, dtype=np.float32)  # pad: no iota match
    dst_val[slot] = (d_o % GW).astype(np.float32)

    # per-call 16-wrap: call = (sg, r) covering len(sg)*SB slots
    cols = []
    pos = 0
    for sg in SGS:
        ncall = len(sg) * SB
        for r_ in range(NR):
            blk = idx_val[pos : pos + ncall]
            cols.append(blk.reshape(ncall // 16, 16).T)  # [16, ncall/16]
            pos += ncall
    idx16 = np.tile(np.concatenate(cols, axis=1), (8, 1))  # [128, TOTSLOTS/16]

    dstS = np.ascontiguousarray(dst_val.reshape(NCHUNKS, 128).T)  # [128, NCHUNKS] f32
    return idx16, dstS


_L1ONLY = bool(int(os.environ.get("GNN_L1ONLY", "0")))
_SKIP_AG = bool(int(os.environ.get("GNN_SKIP_AG", "0")))
_SKIP_GATHER = bool(int(os.environ.get("GNN_SKIP_GATHER", "0")))
_REPEAT = int(os.environ.get("GNN_REPEAT", "1"))


def _build_program(repeat=None):
    import concourse.bass as bass
    import concourse.tile as tile
    from concourse import bacc, mybir

    if repeat is None:
        repeat = _REPEAT
    nc = bacc.Bacc(
        None,
        target_bir_lowering=False,
        num_swdge_queues=4,
        dynamic_dma_scratch_size=SCRATCH,
    )
    dt = mybir.dt

    # inputs
    xbf = nc.dram_tensor("xbf", [N, F], dt.bfloat16, kind="ExternalInput")
    xiT = nc.dram_tensor("xiT", [F, SHARD], dt.bfloat16, kind="ExternalInput")
    idx16 = nc.dram_tensor("idx16", [128, TOTSLOTS // 16], dt.int16, kind="ExternalInput")
    dstS_in = nc.dram_tensor("dstS", [128, NCHUNKS], dt.float32, kind="ExternalInput")
    wr1T = nc.dram_tensor("wr1T", [F, F], dt.bfloat16, kind="ExternalInput")
    wo1T = nc.dram_tensor("wo1T", [F, F], dt.bfloat16, kind="ExternalInput")
    wr2T = nc.dram_tensor("wr2T", [F, O], dt.bfloat16, kind="ExternalInput")
    wo2T = nc.dram_tensor("wo2T", [F, O], dt.bfloat16, kind="ExternalInput")
    b1_in = nc.dram_tensor("b1", [1, F], dt.float32, kind="ExternalInput")
    b2_in = nc.dram_tensor("b2", [1, O], dt.float32, kind="ExternalInput")
    iota_in = nc.dram_tensor("iota", [128, GW], dt.bfloat16, kind="ExternalInput")
    ident_in = nc.dram_tensor("ident", [128, 128], dt.bfloat16, kind="ExternalInput")
    ones_in = nc.dram_tensor("ones", [1, GW], dt.float32, kind="ExternalInput")
    out_t = nc.dram_tensor("out", [SHARD, O], dt.float32, kind="ExternalOutput")

    # internal DRAM: exchange h (128-wide bf16; 256-B rows gather directly)
    h_shard = nc.dram_tensor("h_shard", [SHARD, F], dt.bfloat16)
    h_full = nc.dram_tensor("h_full", [N, F], dt.bfloat16, addr_space="Shared")

    with tile.TileContext(nc) as tc, ExitStack() as ctx:
        const_p = ctx.enter_context(tc.tile_pool(name="const", bufs=1))
        resid_p = ctx.enter_context(tc.tile_pool(name="resid", bufs=1))
        idx_p = ctx.enter_context(tc.tile_pool(name="idxp", bufs=IBUFS))
        msgs_p = ctx.enter_context(tc.tile_pool(name="msgs", bufs=MBUFS))
        s_p = ctx.enter_context(tc.tile_pool(name="sp", bufs=8))
        agg_p = ctx.enter_context(tc.tile_pool(name="aggp", bufs=3))
        hsb_p = ctx.enter_context(tc.tile_pool(name="hsb", bufs=3))
        osb_p = ctx.enter_context(tc.tile_pool(name="osb", bufs=3))
        ps_agg = ctx.enter_context(tc.tile_pool(name="ps_agg", bufs=PSBUFS, space="PSUM"))
        ps_h = ctx.enter_context(tc.tile_pool(name="ps_h", bufs=2, space="PSUM"))
        ps_t = ctx.enter_context(tc.tile_pool(name="ps_t", bufs=1, space="PSUM"))

        # constants / residents
        c_iota = const_p.tile([128, GW], dt.bfloat16)
        nc.sync.dma_start(c_iota[:], iota_in[:])
        c_ident = const_p.tile([128, 128], dt.bfloat16)
        nc.sync.dma_start(c_ident[:], ident_in[:])
        c_ones = const_p.tile([1, GW], dt.float32)
        nc.sync.dma_start(c_ones[:], ones_in[:])
        c_wr1T = const_p.tile([F, F], dt.bfloat16)
        nc.sync.dma_start(c_wr1T[:], wr1T[:])
        c_wo1T = const_p.tile([F, F], dt.bfloat16)
        nc.sync.dma_start(c_wo1T[:], wo1T[:])
        c_wr2T = const_p.tile([F, O], dt.bfloat16)
        nc.sync.dma_start(c_wr2T[:], wr2T[:])
        c_wo2T = const_p.tile([F, O], dt.bfloat16)
        nc.sync.dma_start(c_wo2T[:], wo2T[:])
        c_b1 = const_p.tile([1, F], dt.float32)
        nc.sync.dma_start(c_b1[:], b1_in[:])
        c_b2 = const_p.tile([1, O], dt.float32)
        nc.sync.dma_start(c_b2[:], b2_in[:])
        c_dstS = const_p.tile([128, NCHUNKS], dt.float32)
        nc.sync.dma_start(c_dstS[:], dstS_in[:])
        r_xiT = resid_p.tile([F, SHARD], dt.bfloat16)
        nc.sync.dma_start(r_xiT[:], xiT[:])
        r_hT = resid_p.tile([F, SHARD], dt.bfloat16)  # written in L1, read in L2

        def layer(L):
            """L=1: table=xbf, produce h (hT resident + h_shard DRAM).
            L=2: table=h_full, produce out. Both gathers are 256-B bf16 rows."""
            table = xbf if L == 1 else h_full
            call_idx = 0   # column offset into idx16 (units of 16-wrapped cols)
            chunk_idx = 0  # global chunk counter (dstS column)
            for sg in SGS:
                ng = len(sg)
                call_slots = ng * SB
                call_cols = call_slots // 16
                blocks = call_slots // 128
                msgs = []
                for r_ in range(NR):
                    it = idx_p.tile([128, call_cols], dt.int16, tag="idx")
                    nc.sync.dma_start(
                        it[:], idx16[:, call_idx : call_idx + call_cols]
                    )
                    m = msgs_p.tile([128, blocks * F], dt.bfloat16, tag="m")
                    if _SKIP_GATHER:
                        nc.vector.memset(m[:], 0.0)
                        msgs.append(m)
                        call_idx += call_cols
                        continue
                    nc.gpsimd.dma_gather(
                        m[:].rearrange("p (c e) -> p c e", e=F),
                        table[r_ * RS : (r_ + 1) * RS, :],
                        it[:],
                        call_slots,
                        call_slots,
                        F,
                        single_packet=False,
                        queue_num=r_,
                    )
                    msgs.append(m)
                    call_idx += call_cols
                for gl, g_ in enumerate(sg):
                    ngn = GW if g_ < NGROUP - 1 else LASTG
                    gbase = g_ * GW
                    psum = ps_agg.tile([128, GW], dt.float32, tag="agg", space="PSUM")
                    nmm = NR * CHUNKS_PER_SEG
                    mm = 0
                    for r_ in range(NR):
                        for k in range(CHUNKS_PER_SEG):
                            b = gl * CHUNKS_PER_SEG + k
                            # chunk index in slot layout: (sg, r, g_local, k)
                            ci = chunk_idx + (r_ * ng + gl) * CHUNKS_PER_SEG + k
                            S = s_p.tile([128, GW], dt.bfloat16, tag="S")
                            nc.vector.tensor_scalar(
                                out=S[:],
                                in0=c_iota[:],
                                scalar1=c_dstS[:, ci : ci + 1],
                                scalar2=None,
                                op0=mybir.AluOpType.is_equal,
                            )
                            nc.tensor.matmul(
                                psum[:],
                                lhsT=msgs[r_][:, b * F : (b + 1) * F],
                                rhs=S[:],
                                start=(mm == 0),
                                stop=(mm == nmm - 1),
                            )
                            mm += 1
                    aggT = agg_p.tile([128, GW], dt.bfloat16, tag="aggT")
                    nc.scalar.copy(out=aggT[:], in_=psum[:])
                    if L == 1:
                        ph = ps_h.tile([128, GW], dt.float32, tag="ph", space="PSUM")
                        nc.tensor.matmul(ph[:], lhsT=c_wr1T[:], rhs=aggT[:], start=True, stop=False)
                        nc.tensor.matmul(ph[:, :ngn], lhsT=c_wo1T[:], rhs=r_xiT[:, gbase : gbase + ngn], start=False, stop=False)
                        nc.tensor.matmul(ph[:, :ngn], lhsT=c_b1[:1, :], rhs=c_ones[:1, :ngn], start=False, stop=True)
                        # relu -> hT resident (bf16)
                        nc.scalar.activation(
                            out=r_hT[:, gbase : gbase + ngn],
                            in_=ph[:, :ngn],
                            func=mybir.ActivationFunctionType.Relu,
                        )
                        # transpose -> node-major h (bf16) -> DRAM for AllGather
                        for hb in range(0, ngn, 128):
                            w = min(128, ngn - hb)
                            pt = ps_t.tile([128, 128], dt.bfloat16, tag="pt", space="PSUM")
                            nc.tensor.transpose(pt[:w, :], r_hT[:, gbase + hb : gbase + hb + w], c_ident[:])
                            hsb = hsb_p.tile([128, F], dt.bfloat16, tag="hsb")
                            nc.scalar.copy(out=hsb[:w, :], in_=pt[:w, :])
                            nc.sync.dma_start(h_shard[gbase + hb : gbase + hb + w, :], hsb[:w, :])
                    else:
                        # out[dst, O] = agg2T.T @ w_rel2.T + hT.T @ w_root2.T + b2
                        for hb in range(0, ngn, 128):
                            w = min(128, ngn - hb)
                            po = ps_h.tile([128, O], dt.float32, tag="po", space="PSUM")
                            nc.tensor.matmul(po[:w, :], lhsT=aggT[:, hb : hb + w], rhs=c_wr2T[:], start=True, stop=False)
                            nc.tensor.matmul(po[:w, :], lhsT=r_hT[:, gbase + hb : gbase + hb + w], rhs=c_wo2T[:], start=False, stop=False)
                            nc.tensor.matmul(po[:w, :], lhsT=c_ones[:1, hb : hb + w], rhs=c_b2[:1, :], start=False, stop=True)
                            osb = osb_p.tile([128, O], dt.float32, tag="osb")
                            nc.scalar.copy(out=osb[:w, :], in_=po[:w, :])
                            nc.sync.dma_start(out_t[gbase + hb : gbase + hb + w, :], osb[:w, :])
                chunk_idx += ng * NR * CHUNKS_PER_SEG

        for _rep in range(repeat):
            layer(1)
            if _L1ONLY:
                zo = osb_p.tile([128, O], dt.float32, tag="osb")
                nc.vector.memset(zo[:], 0.0)
                nc.sync.dma_start(out_t[0:128, :], zo[:])
            elif not _SKIP_AG:
                nc.gpsimd.collective_compute(
                    "AllGather",
                    mybir.AluOpType.bypass,
                    replica_groups=[list(range(NC))],
                    ins=[h_shard[:]],
                    outs=[h_full[:]],
                )
            if not _L1ONLY:
                layer(2)

    nc.finalize()
    return nc


_CACHED = {}


def prepare_in_maps(inputs):
    x = np.asarray(inputs["x"], dtype=np.float32)
    edge_index = np.asarray(inputs["edge_index"])
    w_rel1 = np.asarray(inputs["w_rel1"], dtype=np.float32)
    b_rel1 = np.asarray(inputs["b_rel1"], dtype=np.float32)
    w_root1 = np.asarray(inputs["w_root1"], dtype=np.float32)
    w_rel2 = np.asarray(inputs["w_rel2"], dtype=np.float32)
    b_rel2 = np.asarray(inputs["b_rel2"], dtype=np.float32)
    w_root2 = np.asarray(inputs["w_root2"], dtype=np.float32)

    src = edge_index[0].astype(np.int64)
    dst = edge_index[1].astype(np.int64)

    # adapt the per-bucket slot budget to the data (max over cores; SPMD)
    maxc = 0
    for c in range(NC):
        m = (dst >= c * SHARD) & (dst < (c + 1) * SHARD)
        b = ((dst[m] - c * SHARD) // GW) * NR + src[m] // RS
        cnt = np.bincount(b, minlength=NGROUP * NR)
        maxc = max(maxc, int(cnt.max()))
    _set_budget(maxc)

    xbf = x.astype(bf16)
    iota = np.broadcast_to(np.arange(GW, dtype=np.float32), (128, GW)).astype(bf16)
    ident = np.eye(128, dtype=np.float32).astype(bf16)
    ones = np.ones((1, GW), dtype=np.float32)

    in_maps = []
    for c in range(NC):
        m = (dst >= c * SHARD) & (dst < (c + 1) * SHARD)
        idx16, dstS = _prep_core(src[m], dst[m] - c * SHARD)
        in_maps.append(
            {
                "xbf": xbf,
                "xiT": np.ascontiguousarray(x[c * SHARD : (c + 1) * SHARD, :].T).astype(bf16),
                "idx16": idx16,
                "dstS": dstS,
                "wr1T": np.ascontiguousarray(w_rel1.T).astype(bf16),
                "wo1T": np.ascontiguousarray(w_root1.T).astype(bf16),
                "wr2T": np.ascontiguousarray(w_rel2.T).astype(bf16),
                "wo2T": np.ascontiguousarray(w_root2.T).astype(bf16),
                "b1": b_rel1.reshape(1, F),
                "b2": b_rel2.reshape(1, O),
                "iota": iota,
                "ident": ident,
                "ones": ones,
            }
        )
    return in_maps


def get_nc():
    key = ("nc", GW, SB, SG_SIZE)
    if key not in _CACHED:
        _CACHED[key] = _build_program()
    return _CACHED[key]


def kernel(**inputs):
    from concourse.bass_utils import run_bass_kernel_spmd

    in_maps = prepare_in_maps(inputs)
    nc = get_nc()
    res = run_bass_kernel_spmd(nc, in_maps, core_ids=list(range(NC)), trace=False)
    out = np.concatenate([res.results[c]["out"] for c in range(NC)], axis=0)
    return out.astype(np.float32)



# revision 39
# speedup vs baseline: 428.3953x; 1.0069x over previous
"""2-layer GraphConv GNN on 8 trn2 NeuronCores (Bass/Tile).

Strategy (hardcoded for N=100000 nodes, E=1600000 edges, F=128, H=128, O=64):
  - Shard edges by destination node: core c owns dst in [c*12500, (c+1)*12500).
  - Aggregation via PE matmul segment-sum: edges chunked 128 at a time;
    msgs [128 edges, 128 feat] (bf16, gathered via dma_gather) as lhsT,
    one-hot S [128 edges, GW dst-slots] (built on DVE via iota==dst compare)
    as rhs; accumulate into PSUM [128 feat, GW] per GW-dst group (GW=256).
  - Gather: dma_gather (int16 idx) with sources split into 4 ranges of 25000
    rows; 4 SWDGE queues in parallel. Edges laid out in slots grouped by
    (supergroup, src-range, dst-group), dst-sorted, padded to per-bucket
    budgets sized adaptively from the data (max over the 8 cores, so the
    program stays SPMD-uniform; pads gather row 0 with a no-match S value).
  - All weights/operands bf16 on-chip (PSUM accumulates f32); transforms on
    PE from feature-major agg + host-pretransposed x/w; relu on Act.
  - Inter-layer exchange: AllGather of node-major h (bf16, 256-B rows) so
    layer-2 gathers straight from h_full with the same slot machinery.
"""

import os
import numpy as np
import ml_dtypes
from contextlib import ExitStack

N = 100000
F = 128          # input/hidden feature dim
O = 64           # output dim
NC = 8
SHARD = N // NC          # 12500
GW = int(os.environ.get("GNN_GW", "256"))  # dst nodes per psum group
NGROUP = (SHARD + GW - 1) // GW
LASTG = SHARD - (NGROUP - 1) * GW
NR = 4                   # src ranges (int16 gather index limit)
RS = N // NR             # 25000
# slot budget per (group, range) bucket; overridden adaptively from the
# actual per-bucket edge counts in prepare_in_maps (max over the 8 cores,
# rounded up to a multiple of 128).
SB = int(os.environ.get("GNN_SB", "640" if GW == 128 else "1280"))
SG_SIZE = int(os.environ.get("GNN_SG", "4" if GW == 128 else "2"))
MBUFS = int(os.environ.get("GNN_MBUFS", "8"))
PSBUFS = int(os.environ.get("GNN_PSBUFS", "2"))
IBUFS = int(os.environ.get("GNN_IBUFS", "8"))
SCRATCH = int(os.environ.get("GNN_SCRATCH", "16384"))

bf16 = ml_dtypes.bfloat16


def _recalc():
    global CHUNKS_PER_SEG, SGS, NCHUNKS, TOTSLOTS
    CHUNKS_PER_SEG = SB // 128
    sgs = []
    g0 = 0
    while g0 < NGROUP:
        sgs.append(list(range(g0, min(g0 + SG_SIZE, NGROUP))))
        g0 += SG_SIZE
    SGS = sgs
    NCHUNKS = NGROUP * NR * CHUNKS_PER_SEG
    TOTSLOTS = NGROUP * NR * SB


_recalc()


def _set_budget(max_bucket_count):
    """Adapt SB to the data (shared across cores; program is SPMD)."""
    global SB
    want = max(128, -(-int(max_bucket_count) // 128) * 128)
    if want != SB:
        SB = want
        _recalc()
        _CACHED.clear()


def _prep_core(src, dst_local):
    """Slot layout for one core. Returns idx16 [128, TOTSLOTS//16] (int16,
    per-call 16-wrapped+replicated) and dstS [128, NCHUNKS] f32."""
    g = dst_local // GW
    r = src // RS
    bucket = g * NR + r
    order = np.lexsort((dst_local, bucket))
    s_o = src[order]
    d_o = dst_local[order]
    b_o = bucket[order]
    cnt = np.bincount(b_o, minlength=NGROUP * NR)
    if cnt.max() > SB:
        raise RuntimeError(f"bucket overflow: {cnt.max()} > {SB}")

    # slot base per bucket in (sg, r, g_local) call-major order
    slotbase = np.zeros(NGROUP * NR, dtype=np.int64)
    pos = 0
    for sg in SGS:
        for r_ in range(NR):
            for g_ in sg:
                slotbase[g_ * NR + r_] = pos
                pos += SB
    start = np.zeros(NGROUP * NR + 1, dtype=np.int64)
    np.cumsum(cnt, out=start[1:])
    within = np.arange(len(b_o)) - start[b_o]
    slot = slotbase[b_o] + within

    # pad slots gather row 0 of the range (S row is zero, so value unused).
    # (-1 "skip" semantics are only safe for trailing pads; ours are interior.)
    idx_val = np.zeros(TOTSLOTS, dtype=np.int16)
    idx_val[slot] = (s_o - (s_o // RS) * RS).astype(np.int16)
    dst_val = np.full(TOTSLOTS, GW + 72, dtype=np.float32)  # pad: no iota match
    dst_val[slot] = (d_o % GW).astype(np.float32)

    # per-call 16-wrap: call = (sg, r) covering len(sg)*SB slots
    cols = []
    pos = 0
    for sg in SGS:
        ncall = len(sg) * SB
        for r_ in range(NR):
            blk = idx_val[pos : pos + ncall]
            cols.append(blk.reshape(ncall // 16, 16).T)  # [16, ncall/16]
            pos += ncall
    idx16 = np.tile(np.concatenate(cols, axis=1), (8, 1))  # [128, TOTSLOTS/16]

    dstS = np.ascontiguousarray(dst_val.reshape(NCHUNKS, 128).T)  # [128, NCHUNKS] f32
    return idx16, dstS


_L1ONLY = bool(int(os.environ.get("GNN_L1ONLY", "0")))
_SKIP_AG = bool(int(os.environ.get("GNN_SKIP_AG", "0")))
_SKIP_GATHER = bool(int(os.environ.get("GNN_SKIP_GATHER", "0")))
_REPEAT = int(os.environ.get("GNN_REPEAT", "1"))


def _build_program(repeat=None):
    import concourse.bass as bass
    import concourse.tile as tile
    from concourse import bacc, mybir

    if repeat is None:
        repeat = _REPEAT
    nc = bacc.Bacc(
        None,
        target_bir_lowering=False,
        num_swdge_queues=4,
        dynamic_dma_scratch_size=SCRATCH,
    )
    dt = mybir.dt

    # inputs
    xbf = nc.dram_tensor("xbf", [N, F], dt.bfloat16, kind="ExternalInput")
    xiT = nc.dram_tensor("xiT", [F, SHARD], dt.bfloat16, kind="ExternalInput")
    idx16 = nc.dram_tensor("idx16", [128, TOTSLOTS // 16], dt.int16, kind="ExternalInput")
    dstS_in = nc.dram_tensor("dstS", [128, NCHUNKS], dt.float32, kind="ExternalInput")
    wr1T = nc.dram_tensor("wr1T", [F, F], dt.bfloat16, kind="ExternalInput")
    wo1T = nc.dram_tensor("wo1T", [F, F], dt.bfloat16, kind="ExternalInput")
    wr2T = nc.dram_tensor("wr2T", [F, O], dt.bfloat16, kind="ExternalInput")
    wo2T = nc.dram_tensor("wo2T", [F, O], dt.bfloat16, kind="ExternalInput")
    b1_in = nc.dram_tensor("b1", [1, F], dt.float32, kind="ExternalInput")
    b2_in = nc.dram_tensor("b2", [1, O], dt.float32, kind="ExternalInput")
    iota_in = nc.dram_tensor("iota", [128, GW], dt.bfloat16, kind="ExternalInput")
    ident_in = nc.dram_tensor("ident", [128, 128], dt.bfloat16, kind="ExternalInput")
    ones_in = nc.dram_tensor("ones", [1, GW], dt.float32, kind="ExternalInput")
    out_t = nc.dram_tensor("out", [SHARD, O], dt.float32, kind="ExternalOutput")

    # internal DRAM: exchange h (128-wide bf16; 256-B rows gather directly)
    h_shard = nc.dram_tensor("h_shard", [SHARD, F], dt.bfloat16)
    h_full = nc.dram_tensor("h_full", [N, F], dt.bfloat16, addr_space="Shared")

    with tile.TileContext(nc) as tc, ExitStack() as ctx:
        const_p = ctx.enter_context(tc.tile_pool(name="const", bufs=1))
        resid_p = ctx.enter_context(tc.tile_pool(name="resid", bufs=1))
        idx_p = ctx.enter_context(tc.tile_pool(name="idxp", bufs=IBUFS))
        msgs_p = ctx.enter_context(tc.tile_pool(name="msgs", bufs=MBUFS))
        s_p = ctx.enter_context(tc.tile_pool(name="sp", bufs=8))
        agg_p = ctx.enter_context(tc.tile_pool(name="aggp", bufs=3))
        hsb_p = ctx.enter_context(tc.tile_pool(name="hsb", bufs=3))
        osb_p = ctx.enter_context(tc.tile_pool(name="osb", bufs=3))
        ps_agg = ctx.enter_context(tc.tile_pool(name="ps_agg", bufs=PSBUFS, space="PSUM"))
        ps_h = ctx.enter_context(tc.tile_pool(name="ps_h", bufs=2, space="PSUM"))
        ps_t = ctx.enter_context(tc.tile_pool(name="ps_t", bufs=1, space="PSUM"))

        # constants / residents
        c_iota = const_p.tile([128, GW], dt.bfloat16)
        nc.sync.dma_start(c_iota[:], iota_in[:])
        c_ident = const_p.tile([128, 128], dt.bfloat16)
        nc.sync.dma_start(c_ident[:], ident_in[:])
        c_ones = const_p.tile([1, GW], dt.float32)
        nc.sync.dma_start(c_ones[:], ones_in[:])
        c_wr1T = const_p.tile([F, F], dt.bfloat16)
        nc.sync.dma_start(c_wr1T[:], wr1T[:])
        c_wo1T = const_p.tile([F, F], dt.bfloat16)
        nc.sync.dma_start(c_wo1T[:], wo1T[:])
        c_wr2T = const_p.tile([F, O], dt.bfloat16)
        nc.sync.dma_start(c_wr2T[:], wr2T[:])
        c_wo2T = const_p.tile([F, O], dt.bfloat16)
        nc.sync.dma_start(c_wo2T[:], wo2T[:])
        c_b1 = const_p.tile([1, F], dt.float32)
        nc.sync.dma_start(c_b1[:], b1_in[:])
        c_b2 = const_p.tile([1, O], dt.float32)
        nc.sync.dma_start(c_b2[:], b2_in[:])
        c_dstS = const_p.tile([128, NCHUNKS], dt.float32)
        nc.sync.dma_start(c_dstS[:], dstS_in[:])
        r_xiT = resid_p.tile([F, SHARD], dt.bfloat16)
        nc.sync.dma_start(r_xiT[:], xiT[:])
        r_hT = resid_p.tile([F, SHARD], dt.bfloat16)  # written in L1, read in L2

        def layer(L):
            """L=1: table=xbf, produce h (hT resident + h_shard DRAM).
            L=2: table=h_full, produce out. Both gathers are 256-B bf16 rows."""
            table = xbf if L == 1 else h_full
            call_idx = 0   # column offset into idx16 (units of 16-wrapped cols)
            chunk_idx = 0  # global chunk counter (dstS column)
            for sg in SGS:
                ng = len(sg)
                call_slots = ng * SB
                call_cols = call_slots // 16
                blocks = call_slots // 128
                msgs = []
                for r_ in range(NR):
                    it = idx_p.tile([128, call_cols], dt.int16, tag="idx")
                    nc.sync.dma_start(
                        it[:], idx16[:, call_idx : call_idx + call_cols]
                    )
                    m = msgs_p.tile([128, blocks * F], dt.bfloat16, tag="m")
                    if _SKIP_GATHER:
                        nc.vector.memset(m[:], 0.0)
                        msgs.append(m)
                        call_idx += call_cols
                        continue
                    nc.gpsimd.dma_gather(
                        m[:].rearrange("p (c e) -> p c e", e=F),
                        table[r_ * RS : (r_ + 1) * RS, :],
                        it[:],
                        call_slots,
                        call_slots,
                        F,
                        single_packet=False,
                        queue_num=r_,
                    )
                    msgs.append(m)
                    call_idx += call_cols
                for gl, g_ in enumerate(sg):
                    ngn = GW if g_ < NGROUP - 1 else LASTG
                    gbase = g_ * GW
                    psum = ps_agg.tile([128, GW], dt.float32, tag="agg", space="PSUM")
                    nmm = NR * CHUNKS_PER_SEG
                    mm = 0
                    for r_ in range(NR):
                        for k in range(CHUNKS_PER_SEG):
                            b = gl * CHUNKS_PER_SEG + k
                            # chunk index in slot layout: (sg, r, g_local, k)
                            ci = chunk_idx + (r_ * ng + gl) * CHUNKS_PER_SEG + k
                            S = s_p.tile([128, GW], dt.bfloat16, tag="S")
                            nc.vector.tensor_scalar(
                                out=S[:],
                                in0=c_iota[:],
                                scalar1=c_dstS[:, ci : ci + 1],
                                scalar2=None,
                                op0=mybir.AluOpType.is_equal,
                            )
                            nc.tensor.matmul(
                                psum[:],
                                lhsT=msgs[r_][:, b * F : (b + 1) * F],
                                rhs=S[:],
                                start=(mm == 0),
                                stop=(mm == nmm - 1),
                            )
                            mm += 1
                    aggT = agg_p.tile([128, GW], dt.bfloat16, tag="aggT")
                    nc.scalar.copy(out=aggT[:], in_=psum[:])
                    if L == 1:
                        ph = ps_h.tile([128, GW], dt.float32, tag="ph", space="PSUM")
                        nc.tensor.matmul(ph[:], lhsT=c_wr1T[:], rhs=aggT[:], start=True, stop=False)
                        nc.tensor.matmul(ph[:, :ngn], lhsT=c_wo1T[:], rhs=r_xiT[:, gbase : gbase + ngn], start=False, stop=False)
                        nc.tensor.matmul(ph[:, :ngn], lhsT=c_b1[:1, :], rhs=c_ones[:1, :ngn], start=False, stop=True)
                        # relu -> hT resident (bf16)
                        nc.scalar.activation(
                            out=r_hT[:, gbase : gbase + ngn],
                            in_=ph[:, :ngn],
                            func=mybir.ActivationFunctionType.Relu,
                        )
                        # transpose -> node-major h (bf16) -> DRAM for AllGather
                        for hb in range(0, ngn, 128):
                            w = min(128, ngn - hb)
                            pt = ps_t.tile([128, 128], dt.bfloat16, tag="pt", space="PSUM")
                            nc.tensor.transpose(pt[:w, :], r_hT[:, gbase + hb : gbase + hb + w], c_ident[:])
                            hsb = hsb_p.tile([128, F], dt.bfloat16, tag="hsb")
                            nc.scalar.copy(out=hsb[:w, :], in_=pt[:w, :])
                            nc.sync.dma_start(h_shard[gbase + hb : gbase + hb + w, :], hsb[:w, :])
                    else:
                        # out[dst, O] = agg2T.T @ w_rel2.T + hT.T @ w_root2.T + b2
                        for hb in range(0, ngn, 128):
                            w = min(128, ngn - hb)
                            po = ps_h.tile([128, O], dt.float32, tag="po", space="PSUM")
                            nc.tensor.matmul(po[:w, :], lhsT=aggT[:, hb : hb + w], rhs=c_wr2T[:], start=True, stop=False)
                            nc.tensor.matmul(po[:w, :], lhsT=r_hT[:, gbase + hb : gbase + hb + w], rhs=c_wo2T[:], start=False, stop=False)
                            nc.tensor.matmul(po[:w, :], lhsT=c_ones[:1, hb : hb + w], rhs=c_b2[:1, :], start=False, stop=True)
                            osb = osb_p.tile([128, O], dt.float32, tag="osb")
                            nc.scalar.copy(out=osb[:w, :], in_=po[:w, :])
                            nc.sync.dma_start(out_t[gbase + hb : gbase + hb + w, :], osb[:w, :])
                chunk_idx += ng * NR * CHUNKS_PER_SEG

        for _rep in range(repeat):
            layer(1)
            if _L1ONLY:
                zo = osb_p.tile([128, O], dt.float32, tag="osb")
                nc.vector.memset(zo[:], 0.0)
                nc.sync.dma_start(out_t[0:128, :], zo[:])
            elif not _SKIP_AG:
                nc.gpsimd.collective_compute(
                    "AllGather",
                    mybir.AluOpType.bypass,
                    replica_groups=[list(range(NC))],
                    ins=[h_shard[:]],
                    outs=[h_full[:]],
                )
            if not _L1ONLY:
                layer(2)

    nc.finalize()
    return nc


_CACHED = {}


def prepare_in_maps(inputs):
    x = np.asarray(inputs["x"], dtype=np.float32)
    edge_index = np.asarray(inputs["edge_index"])
    w_rel1 = np.asarray(inputs["w_rel1"], dtype=np.float32)
    b_rel1 = np.asarray(inputs["b_rel1"], dtype=np.float32)
    w_root1 = np.asarray(inputs["w_root1"], dtype=np.float32)
    w_rel2 = np.asarray(inputs["w_rel2"], dtype=np.float32)
    b_rel2 = np.asarray(inputs["b_rel2"], dtype=np.float32)
    w_root2 = np.asarray(inputs["w_root2"], dtype=np.float32)

    src = edge_index[0].astype(np.int64)
    dst = edge_index[1].astype(np.int64)

    # adapt the per-bucket slot budget to the data (max over cores; SPMD)
    maxc = 0
    for c in range(NC):
        m = (dst >= c * SHARD) & (dst < (c + 1) * SHARD)
        b = ((dst[m] - c * SHARD) // GW) * NR + src[m] // RS
        cnt = np.bincount(b, minlength=NGROUP * NR)
        maxc = max(maxc, int(cnt.max()))
    _set_budget(maxc)

    xbf = x.astype(bf16)
    iota = np.broadcast_to(np.arange(GW, dtype=np.float32), (128, GW)).astype(bf16)
    ident = np.eye(128, dtype=np.float32).astype(bf16)
    ones = np.ones((1, GW), dtype=np.float32)

    in_maps = []
    for c in range(NC):
        m = (dst >= c * SHARD) & (dst < (c + 1) * SHARD)
        idx16, dstS = _prep_core(src[m], dst[m] - c * SHARD)
        in_maps.append(
            {
                "xbf": xbf,
                "xiT": np.ascontiguousarray(x[c * SHARD : (c + 1) * SHARD, :].T).astype(bf16),
                "idx16": idx16,
                "dstS": dstS,
                "wr1T": np.ascontiguousarray(w_rel1.T).astype(bf16),
                "wo1T": np.ascontiguousarray(w_root1.T).astype(bf16),
                "wr2T": np.ascontiguousarray(w_rel2.T).astype(bf16),
                "wo2T": np.ascontiguousarray(w_root2.T).astype(bf16),
                "b1": b_rel1.reshape(1, F),
                "b2": b_rel2.reshape(1, O),
                "iota": iota,
                "ident": ident,
                "ones": ones,
            }
        )
    return in_maps


def get_nc():
    key = ("nc", GW, SB, SG_SIZE)
    if key not in _CACHED:
        _CACHED[key] = _build_program()
    return _CACHED[key]


def kernel(**inputs):
    from concourse.bass_utils import run_bass_kernel_spmd

    in_maps = prepare_in_maps(inputs)
    nc = get_nc()
    res = run_bass_kernel_spmd(nc, in_maps, core_ids=list(range(NC)), trace=False)
    out = np.concatenate([res.results[c]["out"] for c in range(NC)], axis=0)
    return out.astype(np.float32)



# revision 49
# speedup vs baseline: 1010.3602x; 2.3585x over previous
"""2-layer GraphConv GNN on 8 trn2 NeuronCores (Bass/Tile).

Strategy (hardcoded for N=100000 nodes, E=1600000 edges, F=128, H=128, O=64):
  - Shard edges by destination node: core c owns dst in [c*12500, (c+1)*12500).
  - Aggregation via PE matmul segment-sum: edges chunked 128 at a time;
    msgs [128 edges, 128 feat] (bf16, gathered via dma_gather) as lhsT,
    one-hot S [128 edges, GW dst-slots] (built on DVE via iota==dst compare)
    as rhs; accumulate into PSUM [128 feat, GW] per GW-dst group (GW=256).
  - Gather: dma_gather (int16 idx) with sources split into 4 ranges of 25000
    rows; 4 SWDGE queues in parallel. Edges laid out in slots grouped by
    (supergroup, src-range, dst-group), dst-sorted, padded to per-bucket
    budgets sized adaptively from the data (max over the 8 cores, so the
    program stays SPMD-uniform; pads gather row 0 with a no-match S value).
  - All weights/operands bf16 on-chip (PSUM accumulates f32); transforms on
    PE from feature-major agg + host-pretransposed x/w; relu on Act.
  - Inter-layer exchange: AllGather of node-major h (bf16, 256-B rows) so
    layer-2 gathers straight from h_full with the same slot machinery.
"""

import os
import numpy as np
import ml_dtypes
from contextlib import ExitStack

N = 100000
F = 128          # input/hidden feature dim
O = 64           # output dim
NC = 8
SHARD = N // NC          # 12500
GW = int(os.environ.get("GNN_GW", "256"))  # dst nodes per psum group
NGROUP = (SHARD + GW - 1) // GW
LASTG = SHARD - (NGROUP - 1) * GW
NR = 4                   # src ranges (int16 gather index limit)
RS = N // NR             # 25000
# slot budget per (group, range) bucket; overridden adaptively from the
# actual per-bucket edge counts in prepare_in_maps (max over the 8 cores,
# rounded up to a multiple of 128).
SB = int(os.environ.get("GNN_SB", "640" if GW == 128 else "1280"))
SG_SIZE = int(os.environ.get("GNN_SG", "4" if GW == 128 else "2"))
MBUFS = int(os.environ.get("GNN_MBUFS", "8"))
PSBUFS = int(os.environ.get("GNN_PSBUFS", "2"))
IBUFS = int(os.environ.get("GNN_IBUFS", "8"))
SBUFS = int(os.environ.get("GNN_SBUFS", "8"))    # S-tile pool depth
ABUFS = int(os.environ.get("GNN_ABUFS", "3"))    # aggT/hsb/osb pool depth
PHBUFS = int(os.environ.get("GNN_PHBUFS", "2"))  # transform psum pool depth
SCRATCH = int(os.environ.get("GNN_SCRATCH", "16384"))
BATCHS = bool(int(os.environ.get("GNN_BATCHS", "1")))  # one S-build per segment

bf16 = ml_dtypes.bfloat16


def _recalc():
    global CHUNKS_PER_SEG, SGS, NCHUNKS, TOTSLOTS
    CHUNKS_PER_SEG = SB // 128
    sgs = []
    g0 = 0
    while g0 < NGROUP:
        sgs.append(list(range(g0, min(g0 + SG_SIZE, NGROUP))))
        g0 += SG_SIZE
    SGS = sgs
    NCHUNKS = NGROUP * NR * CHUNKS_PER_SEG
    TOTSLOTS = NGROUP * NR * SB


_recalc()


def _set_budget(max_bucket_count):
    """Adapt SB to the data (shared across cores; program is SPMD)."""
    global SB
    want = max(128, -(-int(max_bucket_count) // 128) * 128)
    if want != SB:
        SB = want
        _recalc()
        _CACHED.clear()


def _prep_core(src, dst_local):
    """Slot layout for one core. Returns idx16 [128, TOTSLOTS//16] (int16,
    per-call 16-wrapped+replicated) and dstS [128, NCHUNKS] f32."""
    g = dst_local // GW
    r = src // RS
    bucket = g * NR + r
    order = np.lexsort((dst_local, bucket))
    s_o = src[order]
    d_o = dst_local[order]
    b_o = bucket[order]
    cnt = np.bincount(b_o, minlength=NGROUP * NR)
    if cnt.max() > SB:
        raise RuntimeError(f"bucket overflow: {cnt.max()} > {SB}")

    # slot base per bucket in (sg, r, g_local) call-major order
    slotbase = np.zeros(NGROUP * NR, dtype=np.int64)
    pos = 0
    for sg in SGS:
        for r_ in range(NR):
            for g_ in sg:
                slotbase[g_ * NR + r_] = pos
                pos += SB
    start = np.zeros(NGROUP * NR + 1, dtype=np.int64)
    np.cumsum(cnt, out=start[1:])
    within = np.arange(len(b_o)) - start[b_o]
    slot = slotbase[b_o] + within

    # pad slots gather row 0 of the range (S row is zero, so value unused).
    # (-1 "skip" semantics are only safe for trailing pads; ours are interior.)
    idx_val = np.zeros(TOTSLOTS, dtype=np.int16)
    idx_val[slot] = (s_o - (s_o // RS) * RS).astype(np.int16)
    dst_val = np.full(TOTSLOTS, GW + 72, dtype=np.float32)  # pad: no iota match
    dst_val[slot] = (d_o % GW).astype(np.float32)

    # per-call 16-wrap: call = (sg, r) covering len(sg)*SB slots
    cols = []
    pos = 0
    for sg in SGS:
        ncall = len(sg) * SB
        for r_ in range(NR):
            blk = idx_val[pos : pos + ncall]
            cols.append(blk.reshape(ncall // 16, 16).T)  # [16, ncall/16]
            pos += ncall
    idx16 = np.tile(np.concatenate(cols, axis=1), (8, 1))  # [128, TOTSLOTS/16]

    dstS = np.ascontiguousarray(dst_val.reshape(NCHUNKS, 128).T)  # [128, NCHUNKS] f32
    return idx16, dstS


_L1ONLY = bool(int(os.environ.get("GNN_L1ONLY", "0")))
_SKIP_AG = bool(int(os.environ.get("GNN_SKIP_AG", "0")))
_SKIP_GATHER = bool(int(os.environ.get("GNN_SKIP_GATHER", "0")))
_REPEAT = int(os.environ.get("GNN_REPEAT", "1"))


def _build_program(repeat=None):
    import concourse.bass as bass
    import concourse.tile as tile
    from concourse import bacc, mybir

    if repeat is None:
        repeat = _REPEAT
    nc = bacc.Bacc(
        None,
        target_bir_lowering=False,
        num_swdge_queues=4,
        dynamic_dma_scratch_size=SCRATCH,
    )
    dt = mybir.dt

    # inputs
    xbf = nc.dram_tensor("xbf", [N, F], dt.bfloat16, kind="ExternalInput")
    xiT = nc.dram_tensor("xiT", [F, SHARD], dt.bfloat16, kind="ExternalInput")
    idx16 = nc.dram_tensor("idx16", [128, TOTSLOTS // 16], dt.int16, kind="ExternalInput")
    dstS_in = nc.dram_tensor("dstS", [128, NCHUNKS], dt.float32, kind="ExternalInput")
    wr1T = nc.dram_tensor("wr1T", [F, F], dt.bfloat16, kind="ExternalInput")
    wo1T = nc.dram_tensor("wo1T", [F, F], dt.bfloat16, kind="ExternalInput")
    wr2T = nc.dram_tensor("wr2T", [F, O], dt.bfloat16, kind="ExternalInput")
    wo2T = nc.dram_tensor("wo2T", [F, O], dt.bfloat16, kind="ExternalInput")
    b1_in = nc.dram_tensor("b1", [1, F], dt.float32, kind="ExternalInput")
    b2_in = nc.dram_tensor("b2", [1, O], dt.float32, kind="ExternalInput")
    iota_in = nc.dram_tensor("iota", [128, GW], dt.bfloat16, kind="ExternalInput")
    ident_in = nc.dram_tensor("ident", [128, 128], dt.bfloat16, kind="ExternalInput")
    ones_in = nc.dram_tensor("ones", [1, GW], dt.float32, kind="ExternalInput")
    out_t = nc.dram_tensor("out", [SHARD, O], dt.float32, kind="ExternalOutput")

    # internal DRAM: exchange h (128-wide bf16; 256-B rows gather directly)
    h_shard = nc.dram_tensor("h_shard", [SHARD, F], dt.bfloat16)
    h_full = nc.dram_tensor("h_full", [N, F], dt.bfloat16, addr_space="Shared")

    with tile.TileContext(nc) as tc, ExitStack() as ctx:
        const_p = ctx.enter_context(tc.tile_pool(name="const", bufs=1))
        resid_p = ctx.enter_context(tc.tile_pool(name="resid", bufs=1))
        idx_p = ctx.enter_context(tc.tile_pool(name="idxp", bufs=IBUFS))
        msgs_p = ctx.enter_context(tc.tile_pool(name="msgs", bufs=MBUFS))
        s_p = ctx.enter_context(tc.tile_pool(name="sp", bufs=SBUFS))
        agg_p = ctx.enter_context(tc.tile_pool(name="aggp", bufs=ABUFS))
        hsb_p = ctx.enter_context(tc.tile_pool(name="hsb", bufs=ABUFS))
        osb_p = ctx.enter_context(tc.tile_pool(name="osb", bufs=ABUFS))
        ps_agg = ctx.enter_context(tc.tile_pool(name="ps_agg", bufs=PSBUFS, space="PSUM"))
        ps_h = ctx.enter_context(tc.tile_pool(name="ps_h", bufs=PHBUFS, space="PSUM"))
        ps_t = ctx.enter_context(tc.tile_pool(name="ps_t", bufs=1, space="PSUM"))

        # constants / residents
        c_iota = const_p.tile([128, GW], dt.bfloat16)
        nc.sync.dma_start(c_iota[:], iota_in[:])
        c_ident = const_p.tile([128, 128], dt.bfloat16)
        nc.sync.dma_start(c_ident[:], ident_in[:])
        c_ones = const_p.tile([1, GW], dt.float32)
        nc.sync.dma_start(c_ones[:], ones_in[:])
        c_wr1T = const_p.tile([F, F], dt.bfloat16)
        nc.sync.dma_start(c_wr1T[:], wr1T[:])
        c_wo1T = const_p.tile([F, F], dt.bfloat16)
        nc.sync.dma_start(c_wo1T[:], wo1T[:])
        c_wr2T = const_p.tile([F, O], dt.bfloat16)
        nc.sync.dma_start(c_wr2T[:], wr2T[:])
        c_wo2T = const_p.tile([F, O], dt.bfloat16)
        nc.sync.dma_start(c_wo2T[:], wo2T[:])
        c_b1 = const_p.tile([1, F], dt.float32)
        nc.sync.dma_start(c_b1[:], b1_in[:])
        c_b2 = const_p.tile([1, O], dt.float32)
        nc.sync.dma_start(c_b2[:], b2_in[:])
        c_dstS = const_p.tile([128, NCHUNKS], dt.float32)
        nc.sync.dma_start(c_dstS[:], dstS_in[:])
        if BATCHS:
            # bf16 copy for the batched tensor-tensor S build (values are
            # <= GW-1 or the GW+72 pad, all bf16-exact for GW <= 256)
            c_dstSb = const_p.tile([128, NCHUNKS], dt.bfloat16)
            nc.scalar.copy(out=c_dstSb[:], in_=c_dstS[:])
        r_xiT = resid_p.tile([F, SHARD], dt.bfloat16)
        nc.sync.dma_start(r_xiT[:], xiT[:])
        r_hT = resid_p.tile([F, SHARD], dt.bfloat16)  # written in L1, read in L2

        def layer(L):
            """L=1: table=xbf, produce h (hT resident + h_shard DRAM).
            L=2: table=h_full, produce out. Both gathers are 256-B bf16 rows."""
            table = xbf if L == 1 else h_full
            call_idx = 0   # column offset into idx16 (units of 16-wrapped cols)
            chunk_idx = 0  # global chunk counter (dstS column)
            for sg in SGS:
                ng = len(sg)
                call_slots = ng * SB
                call_cols = call_slots // 16
                blocks = call_slots // 128
                msgs = []
                for r_ in range(NR):
                    it = idx_p.tile([128, call_cols], dt.int16, tag="idx")
                    nc.sync.dma_start(
                        it[:], idx16[:, call_idx : call_idx + call_cols]
                    )
                    m = msgs_p.tile([128, blocks * F], dt.bfloat16, tag="m")
                    if _SKIP_GATHER:
                        nc.vector.memset(m[:], 0.0)
                        msgs.append(m)
                        call_idx += call_cols
                        continue
                    nc.gpsimd.dma_gather(
                        m[:].rearrange("p (c e) -> p c e", e=F),
                        table[r_ * RS : (r_ + 1) * RS, :],
                        it[:],
                        call_slots,
                        call_slots,
                        F,
                        single_packet=False,
                        queue_num=r_,
                    )
                    msgs.append(m)
                    call_idx += call_cols
                for gl, g_ in enumerate(sg):
                    ngn = GW if g_ < NGROUP - 1 else LASTG
                    gbase = g_ * GW
                    psum = ps_agg.tile([128, GW], dt.float32, tag="agg", space="PSUM")
                    nmm = NR * CHUNKS_PER_SEG
                    if BATCHS:
                        # one DVE op builds S for a whole (group, range) segment
                        stiles = []
                        for r_ in range(NR):
                            ci0 = chunk_idx + (r_ * ng + gl) * CHUNKS_PER_SEG
                            S9 = s_p.tile(
                                [128, CHUNKS_PER_SEG * GW], dt.bfloat16, tag="S"
                            )
                            nc.vector.scalar_tensor_tensor(
                                out=S9[:].rearrange("p (b g) -> p b g", g=GW),
                                in0=c_iota[:]
                                .unsqueeze(1)
                                .broadcast_to([128, CHUNKS_PER_SEG, GW]),
                                scalar=0.0,
                                in1=c_dstSb[:, ci0 : ci0 + CHUNKS_PER_SEG]
                                .unsqueeze(2)
                                .broadcast_to([128, CHUNKS_PER_SEG, GW]),
                                op0=mybir.AluOpType.add,
                                op1=mybir.AluOpType.is_equal,
                            )
                            stiles.append(S9)
                        mm = 0
                        for r_ in range(NR):
                            for k in range(CHUNKS_PER_SEG):
                                b = gl * CHUNKS_PER_SEG + k
                                nc.tensor.matmul(
                                    psum[:],
                                    lhsT=msgs[r_][:, b * F : (b + 1) * F],
                                    rhs=stiles[r_][:, k * GW : (k + 1) * GW],
                                    start=(mm == 0),
                                    stop=(mm == nmm - 1),
                                )
                                mm += 1
                    else:
                        mm = 0
                        for r_ in range(NR):
                            for k in range(CHUNKS_PER_SEG):
                                b = gl * CHUNKS_PER_SEG + k
                                # chunk index in slot layout: (sg, r, g_local, k)
                                ci = chunk_idx + (r_ * ng + gl) * CHUNKS_PER_SEG + k
                                S = s_p.tile([128, GW], dt.bfloat16, tag="S")
                                nc.vector.tensor_scalar(
                                    out=S[:],
                                    in0=c_iota[:],
                                    scalar1=c_dstS[:, ci : ci + 1],
                                    scalar2=None,
                                    op0=mybir.AluOpType.is_equal,
                                )
                                nc.tensor.matmul(
                                    psum[:],
                                    lhsT=msgs[r_][:, b * F : (b + 1) * F],
                                    rhs=S[:],
                                    start=(mm == 0),
                                    stop=(mm == nmm - 1),
                                )
                                mm += 1
                    aggT = agg_p.tile([128, GW], dt.bfloat16, tag="aggT")
                    nc.scalar.copy(out=aggT[:], in_=psum[:])
                    if L == 1:
                        ph = ps_h.tile([128, GW], dt.float32, tag="ph", space="PSUM")
                        nc.tensor.matmul(ph[:], lhsT=c_wr1T[:], rhs=aggT[:], start=True, stop=False)
                        nc.tensor.matmul(ph[:, :ngn], lhsT=c_wo1T[:], rhs=r_xiT[:, gbase : gbase + ngn], start=False, stop=False)
                        nc.tensor.matmul(ph[:, :ngn], lhsT=c_b1[:1, :], rhs=c_ones[:1, :ngn], start=False, stop=True)
                        # relu -> hT resident (bf16)
                        nc.scalar.activation(
                            out=r_hT[:, gbase : gbase + ngn],
                            in_=ph[:, :ngn],
                            func=mybir.ActivationFunctionType.Relu,
                        )
                        # transpose -> node-major h (bf16) -> DRAM for AllGather
                        for hb in range(0, ngn, 128):
                            w = min(128, ngn - hb)
                            pt = ps_t.tile([128, 128], dt.bfloat16, tag="pt", space="PSUM")
                            nc.tensor.transpose(pt[:w, :], r_hT[:, gbase + hb : gbase + hb + w], c_ident[:])
                            hsb = hsb_p.tile([128, F], dt.bfloat16, tag="hsb")
                            nc.scalar.copy(out=hsb[:w, :], in_=pt[:w, :])
                            nc.sync.dma_start(h_shard[gbase + hb : gbase + hb + w, :], hsb[:w, :])
                    else:
                        # out[dst, O] = agg2T.T @ w_rel2.T + hT.T @ w_root2.T + b2
                        for hb in range(0, ngn, 128):
                            w = min(128, ngn - hb)
                            po = ps_h.tile([128, O], dt.float32, tag="po", space="PSUM")
                            nc.tensor.matmul(po[:w, :], lhsT=aggT[:, hb : hb + w], rhs=c_wr2T[:], start=True, stop=False)
                            nc.tensor.matmul(po[:w, :], lhsT=r_hT[:, gbase + hb : gbase + hb + w], rhs=c_wo2T[:], start=False, stop=False)
                            nc.tensor.matmul(po[:w, :], lhsT=c_ones[:1, hb : hb + w], rhs=c_b2[:1, :], start=False, stop=True)
                            osb = osb_p.tile([128, O], dt.float32, tag="osb")
                            nc.scalar.copy(out=osb[:w, :], in_=po[:w, :])
                            nc.sync.dma_start(out_t[gbase + hb : gbase + hb + w, :], osb[:w, :])
                chunk_idx += ng * NR * CHUNKS_PER_SEG

        for _rep in range(repeat):
            layer(1)
            if _L1ONLY:
                zo = osb_p.tile([128, O], dt.float32, tag="osb")
                nc.vector.memset(zo[:], 0.0)
                nc.sync.dma_start(out_t[0:128, :], zo[:])
            elif not _SKIP_AG:
                nc.gpsimd.collective_compute(
                    "AllGather",
                    mybir.AluOpType.bypass,
                    replica_groups=[list(range(NC))],
                    ins=[h_shard[:]],
                    outs=[h_full[:]],
                )
            if not _L1ONLY:
                layer(2)

    nc.finalize()
    return nc


_CACHED = {}


def prepare_in_maps(inputs):
    x = np.asarray(inputs["x"], dtype=np.float32)
    edge_index = np.asarray(inputs["edge_index"])
    w_rel1 = np.asarray(inputs["w_rel1"], dtype=np.float32)
    b_rel1 = np.asarray(inputs["b_rel1"], dtype=np.float32)
    w_root1 = np.asarray(inputs["w_root1"], dtype=np.float32)
    w_rel2 = np.asarray(inputs["w_rel2"], dtype=np.float32)
    b_rel2 = np.asarray(inputs["b_rel2"], dtype=np.float32)
    w_root2 = np.asarray(inputs["w_root2"], dtype=np.float32)

    src = edge_index[0].astype(np.int64)
    dst = edge_index[1].astype(np.int64)

    # adapt the per-bucket slot budget to the data (max over cores; SPMD)
    maxc = 0
    for c in range(NC):
        m = (dst >= c * SHARD) & (dst < (c + 1) * SHARD)
        b = ((dst[m] - c * SHARD) // GW) * NR + src[m] // RS
        cnt = np.bincount(b, minlength=NGROUP * NR)
        maxc = max(maxc, int(cnt.max()))
    _set_budget(maxc)

    xbf = x.astype(bf16)
    iota = np.broadcast_to(np.arange(GW, dtype=np.float32), (128, GW)).astype(bf16)
    ident = np.eye(128, dtype=np.float32).astype(bf16)
    ones = np.ones((1, GW), dtype=np.float32)

    in_maps = []
    for c in range(NC):
        m = (dst >= c * SHARD) & (dst < (c + 1) * SHARD)
        idx16, dstS = _prep_core(src[m], dst[m] - c * SHARD)
        in_maps.append(
            {
                "xbf": xbf,
                "xiT": np.ascontiguousarray(x[c * SHARD : (c + 1) * SHARD, :].T).astype(bf16),
                "idx16": idx16,
                "dstS": dstS,
                "wr1T": np.ascontiguousarray(w_rel1.T).astype(bf16),
                "wo1T": np.ascontiguousarray(w_root1.T).astype(bf16),
                "wr2T": np.ascontiguousarray(w_rel2.T).astype(bf16),
                "wo2T": np.ascontiguousarray(w_root2.T).astype(bf16),
                "b1": b_rel1.reshape(1, F),
                "b2": b_rel2.reshape(1, O),
                "iota": iota,
                "ident": ident,
                "ones": ones,
            }
        )
    return in_maps


def get_nc():
    key = ("nc", GW, SB, SG_SIZE)
    if key not in _CACHED:
        _CACHED[key] = _build_program()
    return _CACHED[key]


def kernel(**inputs):
    from concourse.bass_utils import run_bass_kernel_spmd

    in_maps = prepare_in_maps(inputs)
    nc = get_nc()
    res = run_bass_kernel_spmd(nc, in_maps, core_ids=list(range(NC)), trace=False)
    out = np.concatenate([res.results[c]["out"] for c in range(NC)], axis=0)
    return out.astype(np.float32)

